# revision 48
# baseline (speedup 1.0000x reference)
"""Trainium2 Bass kernel: causal multi-head attention (B=2, N=2048, C=2048, 16 heads).

Sharding: 16 heads split across 8 cores (2 heads/core, tensor parallel).
Each core computes q/k/v projections for its 2 heads, causal attention,
and its partial out-projection y_c = ctx_c @ wo_c.T (bf16 partials).
Host sums partials + bo.

Design (vs the v1 f32r baseline at 431us; this version ~360us):
  - all activations/weights in bf16 (same PE matmul rate as f32r at 1
    cycle/row, 2x DVE rate, half the DMA/SBUF) -- f32 only in PSUM,
    biases, and softmax sums. Final rel err ~4e-3.
  - V^T -> V natural via batched DMA-XBAR 16-bit transposes (one 3D
    instruction per (chunk, head)) instead of 64 PE transposes + copies
  - softmax row sums: E tiles accumulated elementwise on DVE (odd
    k-tiles, incl. the last) and the otherwise-idle Pool/GpSimd engine
    (even k-tiles; memset+add since its copy is slow) into two bf16
    accumulators per head; chains are <= 8 adds deep so bf16 rounding
    noise averages out across the 128-partition exact-f32 ones-matmul
    reduction. 2 ones-matmuls per (b,h,qchunk) replace the baseline's
    per-k-tile ones-matmul (160 -> 32 PE matmuls, -32us PE).
  - causal masking: S/exp/E-acc/AV all skip the fully-masked column
    range of diagonal tiles (never written, never read); the remaining
    128-wide triangle band gets a 0/1 bf16 mask multiply on DVE.
  - software pipelining: each (b,qchunk)'s out-projection is deferred
    and emitted inside the NEXT chunk, between its attention stream and
    the flushed diagonal AV pair + softmax reduction, so the ~8us of
    out-proj matmuls cover the exp/mask/E-accumulator latency of the
    chunk tail. Diagonal AV pairs lag their k-tile by one so the next
    S pair covers the exp->mask DVE chain latency.
  - y partials written/DMA'd bf16; PSUM->SBUF copies alternate DVE and
    Scalar (Pool cannot access PSUM).
  - startup: first weight/x pieces DMA'd at single-c-tile granularity
    in consumption order; wo prefetched mid-phase-1.
"""

import os
import numpy as np
import ml_dtypes

import concourse.bass as bass
import concourse.tile as tile
from concourse import bacc, mybir
from concourse import bass_utils

F32 = mybir.dt.float32
F32R = mybir.dt.float32r
BF16 = mybir.dt.bfloat16
AF = mybir.ActivationFunctionType

# problem dims (hardcoded per contract)
B = 2
N = 2048
C = 2048
HEADS = 16
HD = 128          # head dim
NCORES = 8
HPC = HEADS // NCORES  # heads per core = 2
E = HPC * HD      # per-core projection width = 256
BN = B * N        # 4096
P = 128
CT = C // P       # 16 contraction tiles
NCH = 512         # n-chunk width for projections
NCHUNKS = BN // NCH   # 8
QCW = 512         # q-chunk width in attention
QCHUNKS = N // QCW    # 4 per batch
KT_PER_B = N // P     # 16 k-tiles per batch
TOK_TILES = BN // P   # 32
SCALE = float(HD) ** -0.5

_CACHE = {}


def _build():
    nc = bacc.Bacc(
        "TRN2",
        target_bir_lowering=False,
        debug=False,
        enable_asserts=False,
        num_devices=NCORES,
    )

    xT = nc.dram_tensor("xT", [C, BN], BF16, kind="ExternalInput").ap()
    wqT = nc.dram_tensor("wqT", [C, E], BF16, kind="ExternalInput").ap()
    wkT = nc.dram_tensor("wkT", [C, E], BF16, kind="ExternalInput").ap()
    wvT = nc.dram_tensor("wvT", [C, E], BF16, kind="ExternalInput").ap()
    woT = nc.dram_tensor("woT", [E, C], BF16, kind="ExternalInput").ap()
    bqh = nc.dram_tensor("bqh", [HPC, P], F32, kind="ExternalInput").ap()
    bkh = nc.dram_tensor("bkh", [HPC, P], F32, kind="ExternalInput").ap()
    bvh = nc.dram_tensor("bvh", [HPC, P], F32, kind="ExternalInput").ap()
    masks = nc.dram_tensor("masks", [4, P, QCW], BF16, kind="ExternalInput").ap()
    ones_d = nc.dram_tensor("ones_d", [P, P], BF16, kind="ExternalInput").ap()
    yp = nc.dram_tensor("yp", [BN, C], BF16, kind="ExternalOutput").ap()

    with tile.TileContext(nc) as tc:
        with tc.tile_pool(name="persist", bufs=1) as persist:
            # persistent per-core activations
            qT = persist.tile([P, HPC, B, N], BF16, tag="qT")
            kT = persist.tile([P, HPC, B, N], BF16, tag="kT")
            vN = persist.tile([P, TOK_TILES, E], BF16, tag="vN")
            masks_sb = persist.tile([P, 4, QCW], BF16, tag="masks")
            ones_sb = persist.tile([P, P], BF16, tag="ones")
            wo_sb = persist.tile([P, HPC, C], BF16, tag="wo")

            # ---------------- Phase 1: projections ----------------
            with tc.tile_pool(name="p1w", bufs=1) as wpool, \
                 tc.tile_pool(name="p1x", bufs=12) as xpool, \
                 tc.tile_pool(name="p1vt", bufs=3) as vtpool, \
                 tc.tile_pool(name="p1_ps", bufs=6, space="PSUM") as pps:
                wq_sb = wpool.tile([P, CT, E], BF16, tag="wq")
                wk_sb = wpool.tile([P, CT, E], BF16, tag="wk")
                wv_sb = wpool.tile([P, CT, E], BF16, tag="wv")
                bq_sb = wpool.tile([P, HPC], F32, tag="bq")
                bk_sb = wpool.tile([P, HPC], F32, tag="bk")
                bv_sb = wpool.tile([P, HPC], F32, tag="bv")

                wsrc = [(wq_sb, wqT), (wk_sb, wkT), (wv_sb, wvT)]
                xTr = xT.rearrange("(t p) n -> p t n", p=P)
                # interleave weight pieces and chunk-0 x pieces in
                # consumption order so the first matmuls start ASAP; the
                # first piece is split at single-c-tile granularity and
                # low-priority constants (biases/masks/ones) go afterwards.
                xh0 = []
                wsrcr = [(dst, src.rearrange("(t p) e -> p t e", p=P))
                         for (dst, src) in wsrc]
                # piece 0, single-c-tile halves
                xc0 = xpool.tile([P, 2, NCH], BF16, tag="xc", name="xc0")
                xh0.append(xc0)
                for sub in range(2):
                    for (dst, srcr) in wsrcr:
                        nc.sync.dma_start(
                            dst[:, sub:sub + 1, :], srcr[:, sub:sub + 1, :])
                    nc.sync.dma_start(
                        xc0[:, sub:sub + 1, :], xTr[:, sub:sub + 1, 0:NCH])
                for piece in range(1, 8):
                    for (dst, srcr) in wsrcr:
                        nc.sync.dma_start(
                            dst[:, piece * 2:(piece + 1) * 2, :],
                            srcr[:, piece * 2:(piece + 1) * 2, :],
                        )
                    xc = xpool.tile([P, 2, NCH], BF16, tag="xc")
                    nc.sync.dma_start(
                        xc[:], xTr[:, piece * 2:(piece + 1) * 2, 0:NCH])
                    xh0.append(xc)
                    if piece == 1:
                        nc.sync.dma_start(bq_sb[:], bqh.rearrange("h p -> p h"))
                        nc.sync.dma_start(bk_sb[:], bkh.rearrange("h p -> p h"))
                        nc.sync.dma_start(bv_sb[:], bvh.rearrange("h p -> p h"))
                nc.sync.dma_start(masks_sb[:], masks.rearrange("a p n -> p a n"))
                nc.sync.dma_start(ones_sb[:], ones_d)

                for ch in range(NCHUNKS):
                    b = ch // (N // NCH)
                    nn0 = (ch % (N // NCH)) * NCH  # within-batch token offset
                    n0 = ch * NCH                  # global token offset
                    if ch == 0:
                        xh = xh0
                    else:
                        xh = []
                        for piece in range(8):
                            xc = xpool.tile([P, 2, NCH], BF16, tag="xc")
                            nc.sync.dma_start(
                                xc[:], xTr[:, piece * 2:(piece + 1) * 2,
                                           n0:n0 + NCH])
                            xh.append(xc)
                        if ch == 2:
                            # prefetch out-proj weights mid-phase-1 (early
                            # enough for phase 2, late enough not to delay
                            # the startup-critical x/w pieces)
                            nc.sync.dma_start(
                                wo_sb[:],
                                woT.rearrange("(h p) f -> p h f", p=P))

                    # 6 accumulators (q/k/v x 2 heads); c-tile outer loop so
                    # each x quarter is released after its 2 c-tiles.
                    accs = [pps.tile([P, NCH], F32, tag="pacc",
                                     name=f"pacc_{ch}_{i}")
                            for i in range(3 * HPC)]
                    for ct in range(CT):
                        xq = xh[ct // 2][:, ct % 2, :]
                        for wi, (wsb, _) in enumerate(wsrc):
                            for h in range(HPC):
                                nc.tensor.matmul(
                                    accs[wi * HPC + h][:],
                                    wsb[:, ct, h * HD:(h + 1) * HD],
                                    xq,
                                    start=(ct == 0),
                                    stop=(ct == CT - 1),
                                )

                    for h in range(HPC):
                        nc.scalar.activation(
                            qT[:, h, b, nn0:nn0 + NCH], accs[h][:],
                            AF.Identity, bias=bq_sb[:, h:h + 1], scale=1.0)
                        nc.scalar.activation(
                            kT[:, h, b, nn0:nn0 + NCH], accs[HPC + h][:],
                            AF.Identity, bias=bk_sb[:, h:h + 1], scale=1.0)
                        # v^T with bias -> bf16, then one batched DMA-XBAR
                        # transpose to V natural [tok, d] (4 tiles/instr)
                        vt = vtpool.tile([P, NCH], BF16, tag="vt")
                        nc.scalar.activation(
                            vt[:], accs[2 * HPC + h][:],
                            AF.Identity, bias=bv_sb[:, h:h + 1], scale=1.0)
                        nc.sync.dma_start(
                            vN[:, ch * (NCH // P):(ch + 1) * (NCH // P),
                               h * HD:(h + 1) * HD],
                            vt[:],
                            transpose=True,
                        )

            # ---------------- Phase 2: attention + out-proj ----------------
            with tc.tile_pool(name="p2e", bufs=6) as epool, \
                 tc.tile_pool(name="p2ea", bufs=8) as eapool, \
                 tc.tile_pool(name="p2ctx", bufs=6) as ctxpool, \
                 tc.tile_pool(name="p2sm", bufs=3) as smpool, \
                 tc.tile_pool(name="p2y", bufs=2) as ysbpool, \
                 tc.tile_pool(name="p2s_ps", bufs=2, space="PSUM") as spool, \
                 tc.tile_pool(name="p2c_ps", bufs=2, space="PSUM") as cps, \
                 tc.tile_pool(name="p2sb_ps", bufs=2, space="PSUM") as sbps, \
                 tc.tile_pool(name="p2y_ps", bufs=2, space="PSUM") as yps:

                def outproj(b, qc, ctx_tiles):
                    """out-projection for one (b, qc) block of 512 tokens"""
                    for nt in range(QCW // P):
                        y_sb = ysbpool.tile([P, C], BF16, tag="ysb")
                        for fc in range(C // 512):
                            y_ps = yps.tile([P, 512], F32, tag="yps")
                            for h in range(HPC):
                                nc.tensor.matmul(
                                    y_ps[:],
                                    ctx_tiles[h][:, nt * P:(nt + 1) * P],
                                    wo_sb[:, h, fc * 512:(fc + 1) * 512],
                                    start=(h == 0), stop=(h == HPC - 1),
                                )
                            # PSUM->SBUF bf16 copies: alternate DVE and
                            # Scalar (Pool cannot access PSUM)
                            if fc % 2 == 0:
                                nc.vector.tensor_copy(
                                    y_sb[:, fc * 512:(fc + 1) * 512],
                                    y_ps[:])
                            else:
                                nc.scalar.copy(
                                    y_sb[:, fc * 512:(fc + 1) * 512],
                                    y_ps[:])
                        row0 = b * N + qc * QCW + nt * P
                        nc.sync.dma_start(yp[row0:row0 + P, :], y_sb[:])

                pending = None  # deferred (b, qc, ctx_tiles) outproj
                for b in range(B):
                    for qc in range(QCHUNKS):
                        nkt = 4 * qc + 4  # causal: k-tiles 0..4qc+3
                        # E accumulated over k-tiles into 4 bf16 accumulators
                        # per head (chains <= 4 adds deep keep bf16 rounding
                        # negligible; the f32 partition-sum matmul is exact).
                        # Odd k-tiles (incl. the last) on DVE, even on the
                        # otherwise-idle Pool engine (memset+add: its plain
                        # copy is slow).
                        ctxu = [cps.tile([P, QCW], F32, tag="ctxu",
                                         name=f"ctxu{h}_{b}_{qc}")
                                for h in range(HPC)]
                        eacc = [[eapool.tile([P, QCW], BF16, tag=f"ea{h}{i}",
                                             name=f"ea{h}{i}_{b}_{qc}")
                                 for i in range(2)] for h in range(HPC)]
                        # both heads interleaved kt-major: each S->exp->mask
                        # ->eacc chain is covered by the other head's matmuls
                        def av_pair(kt, ets, off):
                            for h in range(HPC):
                                nc.tensor.matmul(
                                    ctxu[h][:, off:QCW],
                                    vN[:, b * KT_PER_B + kt,
                                       h * HD:(h + 1) * HD],
                                    ets[h][:, off:QCW],
                                    start=(kt == 0), stop=(kt == nkt - 1),
                                )

                        pending_av = None  # diagonal AV lagged one k-tile
                        for kt in range(nkt):
                            a = kt - 4 * qc
                            off = max(0, a) * P  # causal q offset
                            # columns [0, off) are fully masked: S, exp,
                            # E-acc, and AV all skip them (never read)
                            ets = []
                            for h in range(HPC):
                                sps = spool.tile([P, QCW], F32, tag="s")
                                nc.tensor.matmul(
                                    sps[:, off:],
                                    kT[:, h, b, kt * P:(kt + 1) * P],
                                    qT[:, h, b, qc * QCW + off:(qc + 1) * QCW],
                                    start=True, stop=True,
                                )
                                et = epool.tile([P, QCW], BF16, tag="e")
                                nc.scalar.activation(
                                    et[:, off:], sps[:, off:],
                                    AF.Exp, scale=SCALE
                                )
                                ets.append(et)
                            if pending_av is not None:
                                av_pair(*pending_av)
                                pending_av = None
                            for h in range(HPC):
                                et = ets[h]
                                if a >= 0:
                                    # diagonal: 0/1 triangle mask over the
                                    # 128-wide band [off, off+128)
                                    mw = (a + 1) * P
                                    nc.vector.tensor_mul(
                                        et[:, off:mw], et[:, off:mw],
                                        masks_sb[:, a, off:mw]
                                    )
                                ea = eacc[h][kt % 2]
                                if kt % 2 == 1:
                                    if kt < 2:
                                        if off > 0:
                                            nc.vector.memzero(ea[:, :off])
                                        nc.vector.tensor_copy(
                                            ea[:, off:], et[:, off:])
                                    else:
                                        nc.vector.tensor_add(
                                            ea[:, off:], ea[:, off:],
                                            et[:, off:])
                                else:
                                    if kt < 2:
                                        nc.gpsimd.memzero(ea[:])
                                        nc.gpsimd.tensor_add(
                                            ea[:, off:], ea[:, off:],
                                            et[:, off:])
                                    else:
                                        nc.gpsimd.tensor_add(
                                            ea[:, off:], ea[:, off:],
                                            et[:, off:])
                            if a >= 0:
                                # diagonal AV waits on the exp->mask DVE
                                # chain; lag it one k-tile so the next S
                                # pair covers that latency (lagging ALL
                                # tiles measures worse)
                                pending_av = (kt, ets, off)
                            else:
                                av_pair(kt, ets, off)

                        # software pipeline: the PREVIOUS chunk's
                        # out-projection goes here, before the flushed last
                        # AV pair and softmax reduction -- its ~8us of
                        # matmuls cover the exp/mask/E-accumulator latency
                        # of this chunk's tail
                        if pending is not None:
                            outproj(*pending)
                        if pending_av is not None:
                            av_pair(*pending_av)

                        ctx_tiles = []
                        for h in range(HPC):
                            sums_bc = sbps.tile([P, QCW], F32, tag="sumbc")
                            for i in range(2):
                                nc.tensor.matmul(
                                    sums_bc[:], ones_sb[:], eacc[h][i][:],
                                    start=(i == 0), stop=(i == 1),
                                )
                            recip_bc = smpool.tile([P, QCW], F32, tag="recipbc")
                            nc.vector.reciprocal_approx_fast(
                                recip_bc[:], sums_bc[:])
                            ctx = ctxpool.tile([P, QCW], BF16, tag="ctx")
                            nc.vector.tensor_mul(ctx[:], ctxu[h][:], recip_bc[:])
                            ctx_tiles.append(ctx)

                        pending = (b, qc, ctx_tiles)
                outproj(*pending)

    nc.compile()
    return nc


def _host_prep(x, wq, bq, wk, bk, wv, bv, wo):
    """Build the 8 per-core input maps (bf16 data, f32 biases)."""
    bf16 = ml_dtypes.bfloat16
    x = np.asarray(x, dtype=np.float32)
    xT = np.ascontiguousarray(x.reshape(BN, C).T.astype(bf16))  # [C, BN]

    m = np.zeros((4, P, QCW), dtype=np.float32)
    kl = np.arange(P)[:, None]
    ql = np.arange(QCW)[None, :]
    for a in range(4):
        m[a] = (ql >= (P * a + kl)).astype(np.float32)
    m = m.astype(bf16)

    in_maps = []
    for c in range(NCORES):
        e0 = c * E
        in_maps.append({
            "xT": xT,
            "wqT": np.ascontiguousarray(
                np.asarray(wq, np.float32)[e0:e0 + E, :].T.astype(bf16)),
            "wkT": np.ascontiguousarray(
                np.asarray(wk, np.float32)[e0:e0 + E, :].T.astype(bf16)),
            "wvT": np.ascontiguousarray(
                np.asarray(wv, np.float32)[e0:e0 + E, :].T.astype(bf16)),
            "woT": np.ascontiguousarray(
                np.asarray(wo, np.float32)[:, e0:e0 + E].T.astype(bf16)),
            "bqh": np.ascontiguousarray(
                np.asarray(bq, np.float32)[e0:e0 + E].reshape(HPC, P)),
            "bkh": np.ascontiguousarray(
                np.asarray(bk, np.float32)[e0:e0 + E].reshape(HPC, P)),
            "bvh": np.ascontiguousarray(
                np.asarray(bv, np.float32)[e0:e0 + E].reshape(HPC, P)),
            "masks": m,
            "ones_d": np.ones((P, P), dtype=bf16),
        })
    return in_maps


def _ensure_ntff_hook_module():
    """run_bass_kernel_spmd(trace=True) imports antenv.axon_hooks; provide a
    stub (hook=None -> tracing skipped gracefully) if the module is absent."""
    try:
        import antenv.axon_hooks  # noqa: F401
    except ImportError:
        import sys
        import types
        try:
            import antenv
        except ImportError:
            return
        mod = types.ModuleType("antenv.axon_hooks")
        state = {"hook": None}
        mod.set_axon_ntff_profile_hook = lambda h: state.__setitem__("hook", h)
        mod.get_axon_ntff_profile_hook = lambda: state["hook"]
        sys.modules["antenv.axon_hooks"] = mod
        antenv.axon_hooks = mod


def kernel(**inputs):
    _ensure_ntff_hook_module()
    if "nc" not in _CACHE:
        _CACHE["nc"] = _build()
    nc = _CACHE["nc"]

    in_maps = _host_prep(
        inputs["x"], inputs["wq"], inputs["bq"], inputs["wk"], inputs["bk"],
        inputs["wv"], inputs["bv"], inputs["wo"],
    )

    res = bass_utils.run_bass_kernel_spmd(
        nc, in_maps, core_ids=list(range(NCORES)),
        trace=bool(os.environ.get("BASS_TRACE")),
    )
    _CACHE["last_result"] = res

    y = np.zeros((BN, C), dtype=np.float32)
    for c in range(NCORES):
        y += res.results[c]["yp"].astype(np.float32)
    y += np.asarray(inputs["bo"], dtype=np.float32)
    return y.reshape(B, N, C)
